# revision 7
# baseline (speedup 1.0000x reference)
"""Multi-head attention (B=8, N=1024, C=768, 12 heads) on 8 Trainium2 cores.

Strategy: data-parallel over batch — one batch element per NeuronCore, no
collectives. Per core everything stays on-chip:

  1. qkv projection, computed in two orientations:
       - Q^T/K^T tiles [d3, tok]  (lhsT = w_qkv slices, rhs = x^T)
       - V tiles      [tok, d]    (lhsT = x^T slices,  rhs = w_qkv V-columns)
  2. attention per (head, q-half): scores S^T[k, q] = K_h Q_h^T via PE,
     exp on ScalarE (scale=1/8 folded in; no max-subtraction — scores are
     O(5), exp can't overflow), then PV as out^T[hd, q] = V^T-free matmul
     with a ones-column appended to V so row 64 of the PSUM output is the
     softmax denominator. Normalization multiplies rows 0:63 by a
     PE-broadcast reciprocal of row 64.
  3. proj: y[tok, c] = attn_out^T-tiles (lhsT) @ w_proj (+ bias), DMA out.

All matmuls run as float32r (full PE rate at free-dim >= 256, ~1.6e-4 rel
err), accumulation fp32 in PSUM, softmax in fp32.
"""

import numpy as np

import concourse.bass as bass
import concourse.tile as tile
import concourse.mybir as mybir
from concourse import bacc
from concourse.bass_utils import run_bass_kernel_spmd

dt = mybir.dt
AF = mybir.ActivationFunctionType
ALU = mybir.AluOpType

B = 8
C = 768
N = 1024          # tokens per batch element (32*32)
NH = 12           # heads
HD = 64           # head dim
C3 = 3 * C        # 2304
CT = C // 128     # 6 contraction tiles
TT = N // 128     # 8 token tiles
NQH = 2           # q processed in halves of 512
QHW = N // NQH    # 512
SCALE = HD ** -0.5


def _build_nc():
    nc = bacc.Bacc(None, target_bir_lowering=False)

    xt_ext = nc.dram_tensor("xt", [C, N], dt.float32r, kind="ExternalInput")
    wq_ext = nc.dram_tensor("w_qkv", [C, C3], dt.float32r, kind="ExternalInput")
    bqk_ext = nc.dram_tensor("b_qkt", [128, 2 * C // 128], dt.float32, kind="ExternalInput")
    bv_ext = nc.dram_tensor("b_v", [1, C], dt.float32, kind="ExternalInput")
    wp_ext = nc.dram_tensor("w_proj", [C, C], dt.float32r, kind="ExternalInput")
    bp_ext = nc.dram_tensor("b_proj", [1, C], dt.float32, kind="ExternalInput")
    y_ext = nc.dram_tensor("y", [N, C], dt.float32, kind="ExternalOutput")

    with tile.TileContext(nc) as tc, tc.tile_pool(name="persist", bufs=1) as pp:
        with (
            tc.tile_pool(name="xw", bufs=1) as xw,
            tc.tile_pool(name="ps_qk", bufs=4, space="PSUM") as ps_qk,
            tc.tile_pool(name="ps_v", bufs=2, space="PSUM") as ps_v,
        ):
            # ---- constants / biases ----
            ones_f32 = pp.tile([128, NH, 1], dt.float32, tag="ones_f32")
            nc.vector.memset(ones_f32[:], 1.0)
            onesrow_f32 = pp.tile([1, 128], dt.float32, tag="onesrow_f32")
            nc.vector.memset(onesrow_f32[:], 1.0)
            ones_sb = pp.tile([1, 128], dt.float32r, tag="ones")
            nc.vector.tensor_copy(out=ones_sb[:], in_=onesrow_f32[:])
            bqk_sb = pp.tile([128, 2 * C // 128], dt.float32, tag="bqk")
            nc.gpsimd.dma_start(out=bqk_sb[:], in_=bqk_ext[:, :])
            bv_sb = pp.tile([128, C], dt.float32, tag="bv")
            nc.gpsimd.dma_start(out=bv_sb[:], in_=bv_ext[0:1, :].to_broadcast((128, C)))
            bp_sb = pp.tile([128, C], dt.float32, tag="bp")
            nc.gpsimd.dma_start(out=bp_sb[:], in_=bp_ext[0:1, :].to_broadcast((128, C)))
            wp_sb = []
            for i in range(CT):
                t = pp.tile([128, C], dt.float32r, tag=f"wp{i}")
                nc.gpsimd.dma_start(out=t[:], in_=wp_ext[128 * i:128 * (i + 1), :])
                wp_sb.append(t)

            # ---- load x^T and w_qkv ----
            xt_sb = []
            for i in range(CT):
                t = xw.tile([128, N], dt.float32r, tag=f"xt{i}")
                nc.gpsimd.dma_start(out=t[:], in_=xt_ext[128 * i:128 * (i + 1), :])
                xt_sb.append(t)
            wq_sb = []
            for i in range(CT):
                t = xw.tile([128, C3], dt.float32r, tag=f"wq{i}")
                nc.gpsimd.dma_start(out=t[:], in_=wq_ext[128 * i:128 * (i + 1), :])
                wq_sb.append(t)

            # ---- persistent intermediates ----
            # Q^T/K^T: 12 tiles of [128, N] covering d3 rows 0:1536
            qkT = [pp.tile([128, N], dt.float32r, name=f"qkT{i}", tag=f"qkT{i}") for i in range(12)]
            # V (+ones col): per token-tile [128, NH, HD+1]
            v_sb = [pp.tile([128, NH, HD + 1], dt.float32r, name=f"v{i}", tag=f"v{i}") for i in range(TT)]
            # attn_out^T: 6 tiles of [128, N]
            aT = [pp.tile([128, N], dt.float32r, name=f"aT{i}", tag=f"aT{i}") for i in range(CT)]

            # ---- qkv projection: Q^T / K^T part ----
            for d3 in range(12):
                for qh in range(NQH):
                    ps = ps_qk.tile([128, QHW], dt.float32, tag="ps_qk")
                    for ct in range(CT):
                        nc.tensor.matmul(
                            out=ps[:],
                            lhsT=wq_sb[ct][:, 128 * d3:128 * (d3 + 1)],
                            rhs=xt_sb[ct][:, QHW * qh:QHW * (qh + 1)],
                            start=(ct == 0), stop=(ct == CT - 1),
                        )
                    # += bias (per-partition scalar), cast to f32r
                    nc.vector.tensor_scalar(
                        out=qkT[d3][:, QHW * qh:QHW * (qh + 1)],
                        in0=ps[:],
                        scalar1=bqk_sb[:, d3:d3 + 1],
                        scalar2=None,
                        op0=ALU.add,
                    )

            # ---- qkv projection: V part ----
            for tt in range(TT):
                ps = ps_v.tile([128, C], dt.float32, tag="ps_v")
                for half, (c0, c1) in enumerate(((0, 512), (512, C))):
                    for ct in range(CT):
                        nc.tensor.matmul(
                            out=ps[:, c0:c1],
                            lhsT=xt_sb[ct][:, 128 * tt:128 * (tt + 1)],
                            rhs=wq_sb[ct][:, 2 * C + c0:2 * C + c1],
                            start=(ct == 0), stop=(ct == CT - 1),
                        )
                nc.vector.tensor_tensor(
                    out=v_sb[tt][:, :, 0:HD],
                    in0=ps[:].rearrange("p (h d) -> p h d", h=NH),
                    in1=bv_sb[:].rearrange("p (h d) -> p h d", h=NH),
                    op=ALU.add,
                )
                nc.vector.tensor_copy(out=v_sb[tt][:, :, HD:HD + 1], in_=ones_f32[:])

        # ---- attention ----
        with (
            tc.tile_pool(name="att_sb", bufs=3) as att_sb,
            tc.tile_pool(name="att_small", bufs=3) as att_small,
            tc.tile_pool(name="ps_s", bufs=3, space="PSUM") as ps_s,
            tc.tile_pool(name="ps_o", bufs=2, space="PSUM") as ps_o,
            tc.tile_pool(name="ps_r", bufs=2, space="PSUM") as ps_r,
        ):
            for h in range(NH):
                q_tile = qkT[h // 2]
                k_tile = qkT[6 + h // 2]
                po = 64 * (h % 2)
                for qh in range(NQH):
                    qs = slice(QHW * qh, QHW * (qh + 1))
                    pov = ps_o.tile([HD + 1, QHW], dt.float32, tag="pov")
                    for kt in range(TT):
                        pss = ps_s.tile([128, QHW], dt.float32, tag="pss")
                        nc.tensor.matmul(
                            out=pss[:],
                            lhsT=k_tile[po:po + HD, 128 * kt:128 * (kt + 1)],
                            rhs=q_tile[po:po + HD, qs],
                            start=True, stop=True,
                        )
                        es = att_sb.tile([128, QHW], dt.float32r, tag="es")
                        nc.scalar.activation(out=es[:], in_=pss[:], func=AF.Exp, scale=SCALE)
                        nc.tensor.matmul(
                            out=pov[:],
                            lhsT=v_sb[kt][:, h, :],
                            rhs=es[:],
                            start=(kt == 0), stop=(kt == TT - 1),
                        )
                    # normalize rows 0:64 by reciprocal of denominator row 64
                    r_sb = att_small.tile([1, QHW], dt.float32r, tag="r")
                    with nc.allow_low_precision(reason="f32r softmax denom reciprocal"):
                        nc.vector.reciprocal(out=r_sb[:], in_=pov[HD:HD + 1, :])
                    prb = ps_r.tile([128, QHW], dt.float32, tag="prb")
                    nc.tensor.matmul(
                        out=prb[:], lhsT=ones_sb[:], rhs=r_sb[:], start=True, stop=True
                    )
                    rb_sb = att_small.tile([HD, QHW], dt.float32, tag="rb")
                    nc.vector.tensor_copy(out=rb_sb[:], in_=prb[0:HD, :])
                    nc.vector.tensor_tensor(
                        out=aT[h // 2][po:po + HD, qs],
                        in0=pov[0:HD, :],
                        in1=rb_sb[:],
                        op=ALU.mult,
                    )

        # ---- output projection ----
        with (
            tc.tile_pool(name="y_sb", bufs=3) as y_pool,
            tc.tile_pool(name="ps_y", bufs=2, space="PSUM") as ps_y,
        ):
            for tt in range(TT):
                ps = ps_y.tile([128, C], dt.float32, tag="ps_y")
                for c0, c1 in ((0, 512), (512, C)):
                    for ct in range(CT):
                        nc.tensor.matmul(
                            out=ps[:, c0:c1],
                            lhsT=aT[ct][:, 128 * tt:128 * (tt + 1)],
                            rhs=wp_sb[ct][:, c0:c1],
                            start=(ct == 0), stop=(ct == CT - 1),
                        )
                y_sb = y_pool.tile([128, C], dt.float32, tag="y")
                nc.vector.tensor_tensor(out=y_sb[:], in0=ps[:], in1=bp_sb[:], op=ALU.add)
                nc.gpsimd.dma_start(out=y_ext[128 * tt:128 * (tt + 1), :], in_=y_sb[:])

    nc.compile()
    return nc


_NC_CACHE = {}


def kernel(x, w_qkv, b_qkv, w_proj, b_proj, _trace=False):
    x = np.asarray(x, dtype=np.float32)
    w_qkv = np.asarray(w_qkv, dtype=np.float32)
    b_qkv = np.asarray(b_qkv, dtype=np.float32)
    w_proj = np.asarray(w_proj, dtype=np.float32)
    b_proj = np.asarray(b_proj, dtype=np.float32)

    if "nc" not in _NC_CACHE:
        _NC_CACHE["nc"] = _build_nc()
    nc = _NC_CACHE["nc"]

    # host-side prep (pure layout, no arithmetic)
    # b_qkt: Q/K bias columns laid out per d3-tile: [128, 12]
    b_qkt = np.ascontiguousarray(b_qkv[:2 * C].reshape(2 * C // 128, 128).T)
    b_v = np.ascontiguousarray(b_qkv[2 * C:].reshape(1, C))
    b_p = np.ascontiguousarray(b_proj.reshape(1, C))

    core_ids = list(range(B))
    in_maps = []
    for b in range(B):
        xt = np.ascontiguousarray(x[b].reshape(N, C).T)
        in_maps.append({
            "xt": xt,
            "w_qkv": w_qkv,
            "b_qkt": b_qkt,
            "b_v": b_v,
            "w_proj": w_proj,
            "b_proj": b_p,
        })

    res = run_bass_kernel_spmd(nc, in_maps, core_ids, trace=_trace)
    if _trace:
        _NC_CACHE["last_result"] = res

    out = np.empty((B, 32, 32, C), dtype=np.float32)
    for b in range(B):
        out[b] = res.results[b]["y"].reshape(32, 32, C)
    return out
